# revision 5
# baseline (speedup 1.0000x reference)
"""Fused multi-head attention (B=4, N=2048, C=1024, H=16, D=64) on 8 NeuronCores.

Sharding: core i handles batch b = i // 2, head-group g = i % 2 (heads
8g..8g+7).  Each core runs an identical Bass/Tile program (SPMD).

v2 vs v1: the AV matmul is restructured to the "P-stationary" orientation:
  out[q-tile, 0:65] = P[kv, q-tile]^T @ [V | 1]
so each 128x128 exp tile is loaded as PE stationary weights and only 65
moving columns stream per kv tile.  This halves AV streaming cycles
(65 vs 128 per kv-tile per q-tile... strictly: 16*65 vs 2*512 per q-128 per
head) and produces the output directly in natural [token, feature] layout --
no PE transposes, no PSUM->SBUF copies of [65, 2048] stripes.

The exp is split across two engines, by q-columns so every softmax row is
produced by a single engine (any flat per-engine bias cancels in the ratio):
  - ScalarE: table exp for all of head-a tiles and cols [0:SB] of head-b;
  - DVE: Schraudolph bits for head-b cols [SB:1024]:
      bf16_bits = trunc(S * 128*log2(e)*0.125 + (128*127 - 7.334 + 0.5))
    written as uint16 and bitcast to bf16 (+-3% wobble, centered).

AV accumulation per (head, q-tile) runs as two 8-deep PSUM chains (kv
0..7 / 8..15) merged on DVE, so only 4 PSUM banks host 16 in-flight
chains; chains are interleaved into the NEXT half-slot's S matmuls so the
PE never waits on exp.  PSUM: sA 2 + sB 2 + av0..av3 4 = 8 banks.
"""

from contextlib import ExitStack

import ml_dtypes
import numpy as np

import concourse.bass as bass
import concourse.mybir as mybir
import concourse.tile as tile
from concourse import bacc
from concourse.bass_utils import run_bass_kernel_spmd

dt = mybir.dt
AF = mybir.ActivationFunctionType
BF16 = dt.bfloat16
F32 = dt.float32

B, N_TOK, C_IN = 4, 2048, 1024
NH = 8            # heads per core
NPAIR = NH // 2   # head pairs (a=even rows of the K tile, b=odd)
D = 64
WF = 1536         # projected features per core (512 q + 512 k + 512 v)
KC = C_IN // 128  # contraction k-tiles
MT = N_TOK // 128 # kv token tiles
TB = N_TOK // 512 # 512-wide token blocks for the projection
VROW = 65         # V columns per head incl. ones column
SB = 256          # q-cols of each head-b tile exp'd on ScalarE (rest DVE)

K1 = float(128 * np.log2(np.e) * 0.125)
K2 = float(128 * 127 - 128 * 0.0573 + 0.5)  # bf16 bias, Schraudolph centering, trunc


def build_nc(iters: int = 1):
    nc = bacc.Bacc(trn_type="TRN2")
    xT = nc.dram_tensor("xT", [C_IN, N_TOK], BF16, kind="ExternalInput").ap()
    wT = nc.dram_tensor("wT", [C_IN, WF], BF16, kind="ExternalInput").ap()
    qkb = nc.dram_tensor("qkb", [1024], F32, kind="ExternalInput").ap()
    vb = nc.dram_tensor("vb", [512], F32, kind="ExternalInput").ap()
    out = nc.dram_tensor("out", [N_TOK, NH * D], F32, kind="ExternalOutput").ap()

    with tile.TileContext(nc) as tc, ExitStack() as ctx:
        consts = ctx.enter_context(tc.tile_pool(name="consts", bufs=1))
        p_xt = ctx.enter_context(tc.tile_pool(name="p_xt", bufs=KC))
        p_wt = ctx.enter_context(tc.tile_pool(name="p_wt", bufs=KC))
        p_qkt = ctx.enter_context(tc.tile_pool(name="p_qkt", bufs=2 * NPAIR))
        p_vp = ctx.enter_context(tc.tile_pool(name="p_vp", bufs=MT))
        p_pt = ctx.enter_context(tc.tile_pool(name="p_pt", bufs=2))
        p_par = ctx.enter_context(tc.tile_pool(name="p_par", bufs=40))
        p_mrg = ctx.enter_context(tc.tile_pool(name="p_mrg", bufs=4))
        p_eps = ctx.enter_context(tc.tile_pool(name="p_eps", bufs=4))

        qkb_sb = consts.tile([128, 8], F32, name="qkb_sb")
        nc.sync.dma_start(out=qkb_sb, in_=qkb.rearrange("(t p) -> p t", p=128))
        vb_bc = consts.tile([128, 512], F32, name="vb_bc")
        nc.sync.dma_start(
            out=vb_bc,
            in_=bass.AP(tensor=vb.tensor, offset=vb.offset, ap=[[0, 128], vb.ap[0]]),
        )

        def body():
            xt, wt = [], []
            for kc in range(KC):
                tx = p_xt.tile([128, N_TOK], BF16, name=f"xt{kc}", tag="xt")
                nc.sync.dma_start(out=tx, in_=xT[kc * 128 : (kc + 1) * 128, :])
                xt.append(tx)
                tw = p_wt.tile([128, WF], BF16, name=f"wt{kc}", tag="wt")
                nc.sync.dma_start(out=tw, in_=wT[kc * 128 : (kc + 1) * 128, :])
                wt.append(tw)

            qkt = [
                p_qkt.tile([128, N_TOK], BF16, name=f"qkt{ft}", tag="qkt")
                for ft in range(2 * NPAIR)
            ]
            vp = [
                p_vp.tile([128, NH * VROW], BF16, name=f"vp{tt}", tag="vp")
                for tt in range(MT)
            ]

            def proj_qk(ft, pool, tag, after_tb=None, alt=False):
                # qkt[ft] = (x @ w[ft-block].T + b).T  -> [feature, token]
                for tb in range(TB):
                    t = f"{tag}{(ft % 2) * 2 + tb % 2}" if alt else tag
                    ps = pool.tile([128, 512], F32, name=f"pj{ft}_{tb}", tag=t)
                    for kc in range(KC):
                        nc.tensor.matmul(
                            ps,
                            lhsT=wt[kc][:, ft * 128 : (ft + 1) * 128],
                            rhs=xt[kc][:, tb * 512 : (tb + 1) * 512],
                            start=(kc == 0),
                            stop=(kc == KC - 1),
                        )
                    nc.vector.tensor_scalar_add(
                        out=qkt[ft][:, tb * 512 : (tb + 1) * 512],
                        in0=ps,
                        scalar1=qkb_sb[:, ft : ft + 1],
                    )
                    if after_tb is not None:
                        after_tb()

            def proj_v(tt, pool, tag):
                # vp[tt][:, h*65:h*65+64] = x-tile @ w_v[h].T + b_v[h]; col h*65+64 = 1
                ps = pool.tile([128, 512], F32, name=f"pv{tt}", tag=tag)
                for kc in range(KC):
                    nc.tensor.matmul(
                        ps,
                        lhsT=xt[kc][:, tt * 128 : (tt + 1) * 128],
                        rhs=wt[kc][:, 1024:1536],
                        start=(kc == 0),
                        stop=(kc == KC - 1),
                    )
                t = vp[tt]
                nc.gpsimd.memset(t, 1.0)
                for h in range(NH):
                    nc.vector.tensor_add(
                        out=t[:, h * VROW : h * VROW + 64],
                        in0=ps[:, h * 64 : (h + 1) * 64],
                        in1=vb_bc[:, h * 64 : (h + 1) * 64],
                    )

            # ---- phase A: first pair's Q/K projection, then V projection ----
            with tc.tile_pool(name="pp_proj", bufs=4, space="PSUM") as pp_proj:
                proj_qk(0, pp_proj, "pj")
                proj_qk(NPAIR, pp_proj, "pj")
                for tt in range(MT):
                    proj_v(tt, pp_proj, "pj")

            # ---- phase B: attention with P-stationary AV half-chains ----
            with tc.tile_pool(name="pp_s", bufs=1, space="PSUM") as pp_s, \
                 tc.tile_pool(name="pp_av", bufs=1, space="PSUM") as pp_av:

                pending = []       # queued half-chain emitters
                partials = {}      # (head, blk, qt) -> SBUF partial tile
                av_ctr = [0]

                def emit_pending(k):
                    for _ in range(min(k, len(pending))):
                        pending.pop(0)()

                def make_halfchain(n0, hh, qt, head, pt):
                    def emit():
                        av = pp_av.tile(
                            [128, VROW], F32, name="av", tag=f"av{av_ctr[0] % 4}"
                        )
                        av_ctr[0] += 1
                        for mm in range(8):
                            m = hh * 8 + mm
                            nc.tensor.matmul(
                                av,
                                lhsT=pt[:, mm * 1024 + qt * 128 : mm * 1024 + (qt + 1) * 128],
                                rhs=vp[m][:, head * VROW : (head + 1) * VROW],
                                start=(mm == 0),
                                stop=(mm == 7),
                            )
                        key = (head, n0, qt)
                        if hh == 0:
                            par = p_par.tile([128, VROW], F32, name="par", tag="par")
                            nc.vector.tensor_copy(out=par, in_=av)
                            partials[key] = par
                        else:
                            par = partials.pop(key)
                            mg = p_mrg.tile([128, VROW], F32, name="mg", tag="mrg")
                            nc.vector.tensor_add(out=mg, in0=av, in1=par)
                            rc = p_eps.tile([128, 1], F32, name="rc", tag="rc")
                            nc.vector.reciprocal(out=rc, in_=mg[:, 64:65])
                            ob = p_eps.tile([128, 64], F32, name="ob", tag="ob")
                            nc.vector.tensor_scalar_mul(out=ob, in0=mg[:, 0:64], scalar1=rc)
                            q0 = n0 + qt * 128
                            nc.sync.dma_start(
                                out=out[q0 : q0 + 128, head * 64 : (head + 1) * 64],
                                in_=ob,
                            )
                    return emit

                for p in range(NPAIR):
                    ha, hb = 2 * p, 2 * p + 1
                    for blk in range(2):
                        n0 = blk * 1024
                        for hh in range(2):
                            pt_a = p_pt.tile([128, 8 * 1024], BF16, name="pt_a", tag="ptA")
                            pt_b = p_pt.tile([128, 8 * 1024], BF16, name="pt_b", tag="ptB")
                            pt_b_u16 = pt_b.bitcast(dt.uint16)
                            for mm in range(8):
                                m = hh * 8 + mm
                                c0 = mm * 1024
                                for nb in range(2):
                                    s_a = pp_s.tile(
                                        [128, 512], F32, name="s_a", tag="sA", bufs=2
                                    )
                                    s_b = pp_s.tile(
                                        [128, 512], F32, name="s_b", tag="sB", bufs=2
                                    )
                                    nsl = slice(n0 + nb * 512, n0 + (nb + 1) * 512)
                                    nc.tensor.matmul(
                                        s_a,
                                        lhsT=qkt[NPAIR + p][0:64, m * 128 : (m + 1) * 128],
                                        rhs=qkt[p][0:64, nsl],
                                        start=True,
                                        stop=True,
                                    )
                                    nc.tensor.matmul(
                                        s_b,
                                        lhsT=qkt[NPAIR + p][64:128, m * 128 : (m + 1) * 128],
                                        rhs=qkt[p][64:128, nsl],
                                        start=True,
                                        stop=True,
                                    )
                                    cb = c0 + nb * 512
                                    nc.scalar.activation(
                                        out=pt_a[:, cb : cb + 512], in_=s_a,
                                        func=AF.Exp, scale=0.125,
                                    )
                                    if nb == 0:
                                        nc.scalar.activation(
                                            out=pt_b[:, cb : cb + SB], in_=s_b[:, 0:SB],
                                            func=AF.Exp, scale=0.125,
                                        )
                                        nc.vector.tensor_scalar(
                                            out=pt_b_u16[:, cb + SB : cb + 512],
                                            in0=s_b[:, SB:512],
                                            scalar1=K1,
                                            scalar2=K2,
                                            op0=mybir.AluOpType.mult,
                                            op1=mybir.AluOpType.add,
                                        )
                                    else:
                                        nc.vector.tensor_scalar(
                                            out=pt_b_u16[:, cb : cb + 512],
                                            in0=s_b,
                                            scalar1=K1,
                                            scalar2=K2,
                                            op0=mybir.AluOpType.mult,
                                            op1=mybir.AluOpType.add,
                                        )
                                emit_pending(2)
                            for qt in range(8):
                                pending.append(make_halfchain(n0, hh, qt, ha, pt_a))
                                pending.append(make_halfchain(n0, hh, qt, hb, pt_b))

                    # trickle next pair's Q/K projection; sprinkle queued chains
                    if p + 1 < NPAIR:
                        proj_qk(p + 1, pp_av, "av", after_tb=lambda: emit_pending(2), alt=True)
                        proj_qk(NPAIR + p + 1, pp_av, "av", after_tb=lambda: emit_pending(2), alt=True)
                emit_pending(len(pending))

        for _ in range(iters):
            body()

    nc.finalize()
    return nc


_NC_CACHE = {}


def _get_nc(iters: int = 1):
    if iters not in _NC_CACHE:
        _NC_CACHE[iters] = build_nc(iters)
    return _NC_CACHE[iters]


def make_in_maps(x, qkv_w, qkv_b):
    bf = ml_dtypes.bfloat16
    in_maps = []
    for core in range(8):
        b, g = core // 2, core % 2
        xTc = np.ascontiguousarray(x[b].T).astype(bf)
        wq = qkv_w[g * 512 : (g + 1) * 512]
        wk = qkv_w[1024 + g * 512 : 1024 + (g + 1) * 512]
        wv = qkv_w[2048 + g * 512 : 2048 + (g + 1) * 512]
        wTc = np.ascontiguousarray(np.concatenate([wq, wk, wv], axis=0).T).astype(bf)
        qkbc = np.ascontiguousarray(
            np.concatenate(
                [qkv_b[g * 512 : (g + 1) * 512], qkv_b[1024 + g * 512 : 1024 + (g + 1) * 512]]
            )
        ).astype(np.float32)
        vbc = np.ascontiguousarray(qkv_b[2048 + g * 512 : 2048 + (g + 1) * 512]).astype(
            np.float32
        )
        in_maps.append({"xT": xTc, "wT": wTc, "qkb": qkbc, "vb": vbc})
    return in_maps


_RUNNER_CACHE = {}


def _get_runner(iters: int = 1, n_cores: int = 8):
    """Build the shard_map-wrapped bass_exec executable once and reuse it, so
    repeated kernel() calls don't re-ship the NEFF through the axon tunnel."""
    if iters in _RUNNER_CACHE:
        return _RUNNER_CACHE[iters]
    import jax
    from jax.sharding import Mesh, PartitionSpec
    from jax.experimental.shard_map import shard_map
    from concourse.bass2jax import (
        _bass_exec_p,
        install_neuronx_cc_hook,
        partition_id_tensor,
    )

    nc = _get_nc(iters)
    install_neuronx_cc_hook()
    partition_name = nc.partition_id_tensor.name if nc.partition_id_tensor else None
    in_names, out_names, out_avals, zero_outs = [], [], [], []
    for alloc in nc.m.functions[0].allocations:
        if not isinstance(alloc, mybir.MemoryLocationSet):
            continue
        name = alloc.memorylocations[0].name
        if alloc.kind == "ExternalInput":
            if name != partition_name:
                in_names.append(name)
        elif alloc.kind == "ExternalOutput":
            shape = tuple(alloc.tensor_shape)
            npdt = dt.np(alloc.dtype)
            out_names.append(name)
            out_avals.append(jax.core.ShapedArray(shape, npdt))
            zero_outs.append(np.zeros(shape, npdt))
    n_params = len(in_names)
    all_in_names = list(in_names) + list(out_names)
    if partition_name is not None:
        all_in_names.append(partition_name)

    def _body(*args):
        operands = list(args)
        if partition_name is not None:
            operands.append(partition_id_tensor())
        return tuple(
            _bass_exec_p.bind(
                *operands,
                out_avals=tuple(out_avals),
                in_names=tuple(all_in_names),
                out_names=tuple(out_names),
                lowering_input_output_aliases=(),
                sim_require_finite=True,
                sim_require_nnan=True,
                nc=nc,
            )
        )

    devices = jax.devices()[:n_cores]
    mesh = Mesh(np.asarray(devices), ("core",))
    in_specs = (PartitionSpec("core"),) * (n_params + len(out_names))
    out_specs = (PartitionSpec("core"),) * len(out_names)
    fn = jax.jit(
        shard_map(_body, mesh=mesh, in_specs=in_specs, out_specs=out_specs, check_rep=False)
    )
    zero_concat = [
        np.zeros((n_cores * z.shape[0], *z.shape[1:]), z.dtype) for z in zero_outs
    ]
    _RUNNER_CACHE[iters] = (fn, in_names, zero_concat, mesh)
    return _RUNNER_CACHE[iters]


def kernel(x, qkv_w, qkv_b):
    import jax

    x = np.asarray(x, dtype=np.float32)
    qkv_w = np.asarray(qkv_w, dtype=np.float32)
    qkv_b = np.asarray(qkv_b, dtype=np.float32)
    in_maps = make_in_maps(x, qkv_w, qkv_b)
    fn, in_names, zero_concat, _ = _get_runner(1)
    concat_in = [
        np.concatenate([in_maps[c][name] for c in range(8)], axis=0) for name in in_names
    ]
    outs = fn(*concat_in, *zero_concat)
    out_global = np.asarray(jax.block_until_ready(outs)[0])
    full = np.empty((B, N_TOK, C_IN), dtype=np.float32)
    for core in range(8):
        b, g = core // 2, core % 2
        full[b, :, g * 512 : (g + 1) * 512] = out_global[core * N_TOK : (core + 1) * N_TOK]
    return full


# revision 6
# speedup vs baseline: 2.3769x; 2.3769x over previous
"""Fused multi-head attention (B=4, N=2048, C=1024, H=16, D=64) on 8 NeuronCores.

Sharding: core i handles batch b = i // 2, head-group g = i % 2 (heads
8g..8g+7).  Each core runs an identical Bass/Tile program (SPMD).

v2 vs v1: the AV matmul is restructured to the "P-stationary" orientation:
  out[q-tile, 0:65] = P[kv, q-tile]^T @ [V | 1]
so each 128x128 exp tile is loaded as PE stationary weights and only 65
moving columns stream per kv tile.  This halves AV streaming cycles
(65 vs 128 per kv-tile per q-tile... strictly: 16*65 vs 2*512 per q-128 per
head) and produces the output directly in natural [token, feature] layout --
no PE transposes, no PSUM->SBUF copies of [65, 2048] stripes.

The exp is split across two engines, by q-columns so every softmax row is
produced by a single engine (any flat per-engine bias cancels in the ratio):
  - ScalarE: table exp for all of head-a tiles and cols [0:SB] of head-b;
  - DVE: Schraudolph bits for head-b cols [SB:1024]:
      bf16_bits = trunc(S * 128*log2(e)*0.125 + (128*127 - 7.334 + 0.5))
    written as uint16 and bitcast to bf16 (+-3% wobble, centered).

AV accumulation per (head, q-tile) runs as two 8-deep PSUM chains (kv
0..7 / 8..15) merged on DVE, so only 4 PSUM banks host 16 in-flight
chains; chains are interleaved into the NEXT half-slot's S matmuls so the
PE never waits on exp.  PSUM: sA 2 + sB 2 + av0..av3 4 = 8 banks.
"""

from contextlib import ExitStack

import ml_dtypes
import numpy as np

import concourse.bass as bass
import concourse.mybir as mybir
import concourse.tile as tile
from concourse import bacc
from concourse.bass_utils import run_bass_kernel_spmd

dt = mybir.dt
AF = mybir.ActivationFunctionType
BF16 = dt.bfloat16
F32 = dt.float32

B, N_TOK, C_IN = 4, 2048, 1024
NH = 8            # heads per core
NPAIR = NH // 2   # head pairs (a=even rows of the K tile, b=odd)
D = 64
WF = 1536         # projected features per core (512 q + 512 k + 512 v)
KC = C_IN // 128  # contraction k-tiles
MT = N_TOK // 128 # kv token tiles
TB = N_TOK // 512 # 512-wide token blocks for the projection
VROW = 65         # V columns per head incl. ones column
SB = 256          # q-cols of each head-b tile exp'd on ScalarE (rest DVE)

K1 = float(128 * np.log2(np.e) * 0.125)
K2 = float(128 * 127 - 128 * 0.0573 + 0.5)  # bf16 bias, Schraudolph centering, trunc


def build_nc(iters: int = 1):
    nc = bacc.Bacc(trn_type="TRN2")
    xT = nc.dram_tensor("xT", [C_IN, N_TOK], BF16, kind="ExternalInput").ap()
    wT = nc.dram_tensor("wT", [C_IN, WF], BF16, kind="ExternalInput").ap()
    qkb = nc.dram_tensor("qkb", [1024], F32, kind="ExternalInput").ap()
    vb = nc.dram_tensor("vb", [512], F32, kind="ExternalInput").ap()
    out = nc.dram_tensor("out", [N_TOK, NH * D], F32, kind="ExternalOutput").ap()

    with tile.TileContext(nc) as tc, ExitStack() as ctx:
        consts = ctx.enter_context(tc.tile_pool(name="consts", bufs=1))
        p_xt = ctx.enter_context(tc.tile_pool(name="p_xt", bufs=KC))
        p_wt = ctx.enter_context(tc.tile_pool(name="p_wt", bufs=KC))
        p_qkt = ctx.enter_context(tc.tile_pool(name="p_qkt", bufs=2 * NPAIR))
        p_vp = ctx.enter_context(tc.tile_pool(name="p_vp", bufs=MT))
        p_pt = ctx.enter_context(tc.tile_pool(name="p_pt", bufs=2))
        p_par = ctx.enter_context(tc.tile_pool(name="p_par", bufs=40))
        p_mrg = ctx.enter_context(tc.tile_pool(name="p_mrg", bufs=8))
        p_eps = ctx.enter_context(tc.tile_pool(name="p_eps", bufs=8))

        qkb_sb = consts.tile([128, 8], F32, name="qkb_sb")
        nc.sync.dma_start(out=qkb_sb, in_=qkb.rearrange("(t p) -> p t", p=128))
        vb_bc = consts.tile([128, 512], F32, name="vb_bc")
        nc.sync.dma_start(
            out=vb_bc,
            in_=bass.AP(tensor=vb.tensor, offset=vb.offset, ap=[[0, 128], vb.ap[0]]),
        )

        def body():
            xt, wt = [], []
            for kc in range(KC):
                tx = p_xt.tile([128, N_TOK], BF16, name=f"xt{kc}", tag="xt")
                nc.sync.dma_start(out=tx, in_=xT[kc * 128 : (kc + 1) * 128, :])
                xt.append(tx)
                tw = p_wt.tile([128, WF], BF16, name=f"wt{kc}", tag="wt")
                nc.sync.dma_start(out=tw, in_=wT[kc * 128 : (kc + 1) * 128, :])
                wt.append(tw)

            qkt = [
                p_qkt.tile([128, N_TOK], BF16, name=f"qkt{ft}", tag="qkt")
                for ft in range(2 * NPAIR)
            ]
            vp = [
                p_vp.tile([128, NH * VROW], BF16, name=f"vp{tt}", tag="vp")
                for tt in range(MT)
            ]

            def proj_qk(ft, pool, tag, after_tb=None, alt=False):
                # qkt[ft] = (x @ w[ft-block].T + b).T  -> [feature, token]
                for tb in range(TB):
                    t = f"{tag}{(ft % 2) * 2 + tb % 2}" if alt else tag
                    ps = pool.tile([128, 512], F32, name=f"pj{ft}_{tb}", tag=t)
                    for kc in range(KC):
                        nc.tensor.matmul(
                            ps,
                            lhsT=wt[kc][:, ft * 128 : (ft + 1) * 128],
                            rhs=xt[kc][:, tb * 512 : (tb + 1) * 512],
                            start=(kc == 0),
                            stop=(kc == KC - 1),
                        )
                    nc.vector.tensor_scalar_add(
                        out=qkt[ft][:, tb * 512 : (tb + 1) * 512],
                        in0=ps,
                        scalar1=qkb_sb[:, ft : ft + 1],
                    )
                    if after_tb is not None:
                        after_tb()

            def proj_v(tt, pool, tag):
                # vp[tt][:, h*65:h*65+64] = x-tile @ w_v[h].T + b_v[h]; col h*65+64 = 1
                ps = pool.tile([128, 512], F32, name=f"pv{tt}", tag=tag)
                for kc in range(KC):
                    nc.tensor.matmul(
                        ps,
                        lhsT=xt[kc][:, tt * 128 : (tt + 1) * 128],
                        rhs=wt[kc][:, 1024:1536],
                        start=(kc == 0),
                        stop=(kc == KC - 1),
                    )
                t = vp[tt]
                nc.gpsimd.memset(t, 1.0)
                for h in range(NH):
                    nc.vector.tensor_add(
                        out=t[:, h * VROW : h * VROW + 64],
                        in0=ps[:, h * 64 : (h + 1) * 64],
                        in1=vb_bc[:, h * 64 : (h + 1) * 64],
                    )

            # ---- phase A: first pair's Q/K projection, then V projection ----
            with tc.tile_pool(name="pp_proj", bufs=4, space="PSUM") as pp_proj:
                proj_qk(0, pp_proj, "pj")
                proj_qk(NPAIR, pp_proj, "pj")
                for tt in range(MT):
                    proj_v(tt, pp_proj, "pj")

            # ---- phase B: attention with P-stationary AV half-chains ----
            with tc.tile_pool(name="pp_s", bufs=1, space="PSUM") as pp_s, \
                 tc.tile_pool(name="pp_av", bufs=1, space="PSUM") as pp_av:

                pending = []       # queued half-chain emitters
                partials = {}      # (head, blk, qt) -> SBUF partial tile
                av_ctr = [0]

                def emit_pending(k):
                    for _ in range(min(k, len(pending))):
                        pending.pop(0)()

                def make_halfchain(n0, hh, qt, head, pt):
                    def emit():
                        av = pp_av.tile(
                            [128, VROW], F32, name="av", tag=f"av{av_ctr[0] % 4}"
                        )
                        av_ctr[0] += 1
                        for mm in range(8):
                            m = hh * 8 + mm
                            nc.tensor.matmul(
                                av,
                                lhsT=pt[:, mm * 1024 + qt * 128 : mm * 1024 + (qt + 1) * 128],
                                rhs=vp[m][:, head * VROW : (head + 1) * VROW],
                                start=(mm == 0),
                                stop=(mm == 7),
                            )
                        key = (head, n0, qt)
                        if hh == 0:
                            par = p_par.tile([128, VROW], F32, name="par", tag="par")
                            nc.vector.tensor_copy(out=par, in_=av)
                            partials[key] = par
                        else:
                            par = partials.pop(key)
                            mg = p_mrg.tile([128, VROW], F32, name="mg", tag="mrg")
                            nc.vector.tensor_add(out=mg, in0=av, in1=par)
                            rc = p_eps.tile([128, 1], F32, name="rc", tag="rc")
                            nc.vector.reciprocal(out=rc, in_=mg[:, 64:65])
                            ob = p_eps.tile([128, 64], F32, name="ob", tag="ob")
                            nc.vector.tensor_scalar_mul(out=ob, in0=mg[:, 0:64], scalar1=rc)
                            q0 = n0 + qt * 128
                            nc.sync.dma_start(
                                out=out[q0 : q0 + 128, head * 64 : (head + 1) * 64],
                                in_=ob,
                            )
                    return emit

                for p in range(NPAIR):
                    ha, hb = 2 * p, 2 * p + 1
                    for blk in range(2):
                        n0 = blk * 1024
                        for hh in range(2):
                            pt_a = p_pt.tile([128, 8 * 1024], BF16, name="pt_a", tag="ptA")
                            pt_b = p_pt.tile([128, 8 * 1024], BF16, name="pt_b", tag="ptB")
                            pt_b_u16 = pt_b.bitcast(dt.uint16)
                            for mm in range(8):
                                m = hh * 8 + mm
                                c0 = mm * 1024
                                for nb in range(2):
                                    s_a = pp_s.tile(
                                        [128, 512], F32, name="s_a", tag="sA", bufs=2
                                    )
                                    s_b = pp_s.tile(
                                        [128, 512], F32, name="s_b", tag="sB", bufs=2
                                    )
                                    nsl = slice(n0 + nb * 512, n0 + (nb + 1) * 512)
                                    nc.tensor.matmul(
                                        s_a,
                                        lhsT=qkt[NPAIR + p][0:64, m * 128 : (m + 1) * 128],
                                        rhs=qkt[p][0:64, nsl],
                                        start=True,
                                        stop=True,
                                    )
                                    nc.tensor.matmul(
                                        s_b,
                                        lhsT=qkt[NPAIR + p][64:128, m * 128 : (m + 1) * 128],
                                        rhs=qkt[p][64:128, nsl],
                                        start=True,
                                        stop=True,
                                    )
                                    cb = c0 + nb * 512
                                    nc.scalar.activation(
                                        out=pt_a[:, cb : cb + 512], in_=s_a,
                                        func=AF.Exp, scale=0.125,
                                    )
                                    if nb == 0:
                                        nc.scalar.activation(
                                            out=pt_b[:, cb : cb + SB], in_=s_b[:, 0:SB],
                                            func=AF.Exp, scale=0.125,
                                        )
                                        nc.vector.tensor_scalar(
                                            out=pt_b_u16[:, cb + SB : cb + 512],
                                            in0=s_b[:, SB:512],
                                            scalar1=K1,
                                            scalar2=K2,
                                            op0=mybir.AluOpType.mult,
                                            op1=mybir.AluOpType.add,
                                        )
                                    else:
                                        nc.vector.tensor_scalar(
                                            out=pt_b_u16[:, cb : cb + 512],
                                            in0=s_b,
                                            scalar1=K1,
                                            scalar2=K2,
                                            op0=mybir.AluOpType.mult,
                                            op1=mybir.AluOpType.add,
                                        )
                                emit_pending(2)
                            for qt in range(8):
                                pending.append(make_halfchain(n0, hh, qt, ha, pt_a))
                                pending.append(make_halfchain(n0, hh, qt, hb, pt_b))

                    # trickle next pair's Q/K projection; sprinkle queued chains
                    if p + 1 < NPAIR:
                        proj_qk(p + 1, pp_av, "av", after_tb=lambda: emit_pending(2), alt=True)
                        proj_qk(NPAIR + p + 1, pp_av, "av", after_tb=lambda: emit_pending(2), alt=True)
                emit_pending(len(pending))

        for _ in range(iters):
            body()

    nc.finalize()
    return nc


_NC_CACHE = {}


def _get_nc(iters: int = 1):
    if iters not in _NC_CACHE:
        _NC_CACHE[iters] = build_nc(iters)
    return _NC_CACHE[iters]


def make_in_maps(x, qkv_w, qkv_b):
    bf = ml_dtypes.bfloat16
    in_maps = []
    for core in range(8):
        b, g = core // 2, core % 2
        xTc = np.ascontiguousarray(x[b].T).astype(bf)
        wq = qkv_w[g * 512 : (g + 1) * 512]
        wk = qkv_w[1024 + g * 512 : 1024 + (g + 1) * 512]
        wv = qkv_w[2048 + g * 512 : 2048 + (g + 1) * 512]
        wTc = np.ascontiguousarray(np.concatenate([wq, wk, wv], axis=0).T).astype(bf)
        qkbc = np.ascontiguousarray(
            np.concatenate(
                [qkv_b[g * 512 : (g + 1) * 512], qkv_b[1024 + g * 512 : 1024 + (g + 1) * 512]]
            )
        ).astype(np.float32)
        vbc = np.ascontiguousarray(qkv_b[2048 + g * 512 : 2048 + (g + 1) * 512]).astype(
            np.float32
        )
        in_maps.append({"xT": xTc, "wT": wTc, "qkb": qkbc, "vb": vbc})
    return in_maps


_RUNNER_CACHE = {}


def _get_runner(iters: int = 1, n_cores: int = 8):
    """Build the shard_map-wrapped bass_exec executable once and reuse it, so
    repeated kernel() calls don't re-ship the NEFF through the axon tunnel."""
    if iters in _RUNNER_CACHE:
        return _RUNNER_CACHE[iters]
    import jax
    from jax.sharding import Mesh, PartitionSpec
    from jax.experimental.shard_map import shard_map
    from concourse.bass2jax import (
        _bass_exec_p,
        install_neuronx_cc_hook,
        partition_id_tensor,
    )

    nc = _get_nc(iters)
    install_neuronx_cc_hook()
    partition_name = nc.partition_id_tensor.name if nc.partition_id_tensor else None
    in_names, out_names, out_avals, zero_outs = [], [], [], []
    for alloc in nc.m.functions[0].allocations:
        if not isinstance(alloc, mybir.MemoryLocationSet):
            continue
        name = alloc.memorylocations[0].name
        if alloc.kind == "ExternalInput":
            if name != partition_name:
                in_names.append(name)
        elif alloc.kind == "ExternalOutput":
            shape = tuple(alloc.tensor_shape)
            npdt = dt.np(alloc.dtype)
            out_names.append(name)
            out_avals.append(jax.core.ShapedArray(shape, npdt))
            zero_outs.append(np.zeros(shape, npdt))
    n_params = len(in_names)
    all_in_names = list(in_names) + list(out_names)
    if partition_name is not None:
        all_in_names.append(partition_name)

    def _body(*args):
        operands = list(args)
        if partition_name is not None:
            operands.append(partition_id_tensor())
        return tuple(
            _bass_exec_p.bind(
                *operands,
                out_avals=tuple(out_avals),
                in_names=tuple(all_in_names),
                out_names=tuple(out_names),
                lowering_input_output_aliases=(),
                sim_require_finite=True,
                sim_require_nnan=True,
                nc=nc,
            )
        )

    devices = jax.devices()[:n_cores]
    mesh = Mesh(np.asarray(devices), ("core",))
    in_specs = (PartitionSpec("core"),) * (n_params + len(out_names))
    out_specs = (PartitionSpec("core"),) * len(out_names)
    fn = jax.jit(
        shard_map(_body, mesh=mesh, in_specs=in_specs, out_specs=out_specs, check_rep=False)
    )
    zero_concat = [
        np.zeros((n_cores * z.shape[0], *z.shape[1:]), z.dtype) for z in zero_outs
    ]
    _RUNNER_CACHE[iters] = (fn, in_names, zero_concat, mesh)
    return _RUNNER_CACHE[iters]


def kernel(x, qkv_w, qkv_b):
    import jax

    x = np.asarray(x, dtype=np.float32)
    qkv_w = np.asarray(qkv_w, dtype=np.float32)
    qkv_b = np.asarray(qkv_b, dtype=np.float32)
    in_maps = make_in_maps(x, qkv_w, qkv_b)
    fn, in_names, zero_concat, _ = _get_runner(1)
    concat_in = [
        np.concatenate([in_maps[c][name] for c in range(8)], axis=0) for name in in_names
    ]
    outs = fn(*concat_in, *zero_concat)
    out_global = np.asarray(jax.block_until_ready(outs)[0])
    full = np.empty((B, N_TOK, C_IN), dtype=np.float32)
    for core in range(8):
        b, g = core // 2, core % 2
        full[b, :, g * 512 : (g + 1) * 512] = out_global[core * N_TOK : (core + 1) * N_TOK]
    return full
